# revision 9
# baseline (speedup 1.0000x reference)
"""APPNP GNN kernel for 8 TRN2 NeuronCores (Bass/Tile).

Strategy: node-sharded (12500 nodes/core), edges partitioned by dst core.
Per propagation step: AllGather of g = dis*h (bf16 table, 256B rows), then
dma_gather of per-edge source rows, weighted segment-sum via two matmul
levels (transposed mm1 routes tokens->16-slot windows in PSUM free axis;
PE transpose; mm2 routes slots->window rows with host-built selector
matrices), accumulated into SBUF agg. Degrees computed on device by the
same pipeline with an all-ones rhs. Epilogue pools by graph id via matmul
with a host-built one-hot, AllReduce, linear layer + log_softmax.
"""
import sys
import types

sys.path.insert(0, "/opt/trn_rl_repo")

import numpy as np

N = 100000
E = 3200000
F_IN = 128
HID = 64
N_CLASSES = 10
N_GRAPHS = 512
K = 5
ALPHA = 0.2
NC_ = 8
NPC = N // NC_          # 12500 nodes per core
NW = 98                 # windows of 128 rows
NPCP = NW * 128         # 12544 padded rows per core
NRANGE = 2 * NPCP       # 25088 rows per int16 index range
NROWS = NC_ * NPCP      # 100352 table rows
SPAN = 16               # max distinct nodes per 128-token tile
MAXCHUNK = 8            # tiles per gather call (<=1024 idxs)

_CACHE = {}


def _build_structures(edge_index, edge_weight, batch):
    import ml_dtypes

    BF16 = ml_dtypes.bfloat16
    src = np.asarray(edge_index[0], dtype=np.int64)
    dst = np.asarray(edge_index[1], dtype=np.int64)
    w = np.asarray(edge_weight, dtype=np.float32)
    batch = np.asarray(batch, dtype=np.int64)

    prow = (src // NPC) * NPCP + (src % NPC)
    rho_all = prow // NRANGE
    idx16_all = (prow - rho_all * NRANGE).astype(np.int16)
    core_all = dst // NPC
    ldst_all = (dst - core_all * NPC).astype(np.int64)

    # per (core, rho, win): tile lists with span<=SPAN rule
    per_core = []
    tiles_need = np.zeros((NC_, 4, NW), np.int64)
    for c in range(NC_):
        sel = np.nonzero(core_all == c)[0]
        ld = ldst_all[sel]
        rh = rho_all[sel]
        wi = ld // 128
        order = np.lexsort((ld, wi, rh))
        sel = sel[order]
        ld = ld[order]
        rh = rh[order]
        wi = wi[order]
        # segment boundaries for (rho, win)
        key = rh * NW + wi
        bounds = np.nonzero(np.diff(key))[0] + 1
        seg_starts = np.concatenate([[0], bounds])
        seg_ends = np.concatenate([bounds, [len(key)]])
        segs = {}
        for s, e in zip(seg_starts, seg_ends):
            r, ww = int(rh[s]), int(wi[s])
            # tile walk: each tile: up to 128 tokens, node span < SPAN
            tl = []
            p = s
            while p < e:
                base = int(ld[p])
                lim = p + np.searchsorted(ld[p:e], base + SPAN, side="left")
                q = min(p + 128, int(lim), e)
                tl.append((p, q, base))
                p = q
            segs[(r, ww)] = tl
            tiles_need[c, r, ww] = len(tl)
        per_core.append((sel, ld, segs))

    T = tiles_need.max(axis=0)  # [4, NW] uniform tiles per phase
    T = np.maximum(T, 1)
    # schedule: phases in (rho, win) order
    phases = []  # (rho, win, ntiles, tile0)
    t0 = 0
    for r in range(4):
        for ww in range(NW):
            phases.append((r, ww, int(T[r, ww]), t0))
            t0 += int(T[r, ww])
    ntiles = t0
    ntok = ntiles * 128
    # gather chunks per rho (cannot cross rho boundary)
    chunks = []  # (tile0, ntiles)
    for r in range(4):
        a = sum(int(T[rr, ww]) for rr in range(r) for ww in range(NW))
        b = a + sum(int(T[r, ww]) for ww in range(NW))
        p = a
        while p < b:
            nt = min(MAXCHUNK, b - p)
            chunks.append((p, nt))
            p += nt

    # per-core token arrays
    idx_rep_all, gw_all, sa_all, sb_all = [], [], [], []
    for c in range(NC_):
        sel, ld, segs = per_core[c]
        tok_idx = np.zeros(ntok, np.int16)
        tok_w = np.zeros(ntok, np.float32)
        tok_m = np.zeros(ntok, np.int64)
        tile_base = np.zeros(ntiles, np.int64)  # window-local base row of tile
        for r, ww, nt, tile0 in phases:
            tl = segs.get((r, ww), [])
            for k in range(nt):
                gt = tile0 + k
                if k < len(tl):
                    s, e, base = tl[k]
                    n = e - s
                    pos = gt * 128 + np.arange(n)
                    eidx = sel[s:e]
                    tok_idx[pos] = idx16_all[eidx]
                    tok_w[pos] = w[eidx]
                    tok_m[pos] = ld[s:e] - base
                    tile_base[gt] = base - ww * 128
                else:
                    tile_base[gt] = 0
        # wrapped idx layout [16, ntok/16] replicated x8
        iw = tok_idx.reshape(ntok // 16, 16).T
        idx_rep = np.broadcast_to(iw[None], (8, 16, ntok // 16)).reshape(128, ntok // 16)
        idx_rep_all.append(np.ascontiguousarray(idx_rep))
        # G_w [ntiles, 128, 16]
        gw = np.zeros((ntiles, 128, SPAN), np.float32)
        allpos = np.arange(ntok)
        gw[allpos // 128, allpos % 128, tok_m] = tok_w
        gw_all.append(np.ascontiguousarray(gw.transpose(1, 0, 2).reshape(128, ntiles * SPAN)).astype(BF16))
        # S matrices per phase: SA [nph, 128, 128], SB [nph, 32, 128]
        nph = len(phases)
        sa = np.zeros((nph, 128, 128), np.float32)
        sb = np.zeros((nph, 32, 128), np.float32)
        for pi, (r, ww, nt, tile0) in enumerate(phases):
            for k in range(nt):
                gt = tile0 + k
                slot0 = 16 * k
                base = tile_base[gt]
                for m in range(SPAN):
                    row = base + m
                    if row < 128:
                        if slot0 + m < 128:
                            sa[pi, slot0 + m, row] = 1.0
                        else:
                            sb[pi, slot0 + m - 128, row] = 1.0
        sa_all.append(sa.astype(BF16))
        sb_all.append(sb.astype(BF16))

    # pooling one-hot per core: [128, NW*512]
    poh_all = []
    cnt_mask_all = []
    for c in range(NC_):
        g_ids = batch[c * NPC : (c + 1) * NPC]
        poh = np.zeros((NPCP, N_GRAPHS), np.float32)
        poh[np.arange(NPC), g_ids] = 1.0
        poh = poh.reshape(NW, 128, N_GRAPHS).transpose(1, 0, 2).reshape(128, NW * N_GRAPHS)
        poh_all.append(poh.astype(BF16))

    meta = dict(phases=phases, chunks=chunks, ntiles=ntiles, ntok=ntok)
    arrays = dict(idx=idx_rep_all, gw=gw_all, sa=sa_all, sb=sb_all, poh=poh_all)
    return meta, arrays


def _patch_bass_elem128():
    import concourse.bass as bassmod

    if getattr(bassmod, "_elem128_patched", False):
        return
    src = open(bassmod.__file__).read().replace(
        "elem_size_bytes > 0 and elem_size_bytes % 256 == 0",
        "elem_size_bytes > 0 and elem_size_bytes % 128 == 0",
    )
    exec(compile(src, bassmod.__file__, "exec"), bassmod.__dict__)
    bassmod._elem128_patched = True


def _build_program(meta):
    import ml_dtypes

    _patch_bass_elem128()
    from concourse import bass, bacc, mybir
    from concourse.tile import TileContext
    from concourse.masks import make_identity

    phases = meta["phases"]
    chunks = meta["chunks"]
    ntiles = meta["ntiles"]
    ntok = meta["ntok"]
    nph = len(phases)
    FP32 = mybir.dt.float32
    BF = mybir.dt.bfloat16

    nc = bacc.Bacc("TRN2", num_swdge_queues=4)
    xsh = nc.declare_dram_parameter("xsh", [NPCP, F_IN], FP32, isOutput=False)
    idxp = nc.declare_dram_parameter("idxp", [128, ntok // 16], mybir.dt.int16, isOutput=False)
    gwp = nc.declare_dram_parameter("gwp", [128, ntiles * SPAN], BF, isOutput=False)
    sap = nc.declare_dram_parameter("sap", [nph, 128, 128], BF, isOutput=False)
    sbp = nc.declare_dram_parameter("sbp", [nph, 32, 128], BF, isOutput=False)
    pohp = nc.declare_dram_parameter("pohp", [128, NW * N_GRAPHS], BF, isOutput=False)
    w1p = nc.declare_dram_parameter("w1p", [F_IN, HID], FP32, isOutput=False)
    b1p = nc.declare_dram_parameter("b1p", [1, HID], FP32, isOutput=False)
    w2p = nc.declare_dram_parameter("w2p", [HID, N_CLASSES], FP32, isOutput=False)
    b2p = nc.declare_dram_parameter("b2p", [N_CLASSES, 1], FP32, isOutput=False)
    outp = nc.declare_dram_parameter("out", [N_GRAPHS, N_CLASSES], FP32, isOutput=True)

    tin = nc.dram_tensor("tin", [NPCP, 128], BF)
    table = nc.dram_tensor("table", [NROWS, 128], BF, addr_space="Shared")
    arin = nc.dram_tensor("arin", [HID + 1, N_GRAPHS], FP32)
    arout = nc.dram_tensor("arout", [HID + 1, N_GRAPHS], FP32, addr_space="Shared")
    rtmp = nc.dram_tensor("rtmp", [1, N_GRAPHS], FP32)

    RG = [list(range(NC_))]

    with TileContext(nc) as tc:
        with (
            tc.tile_pool(name="const", bufs=1) as cp,
            tc.tile_pool(name="state", bufs=1) as st,
            tc.tile_pool(name="stream", bufs=3) as sp,
            tc.tile_pool(name="gather", bufs=10) as gp,
            tc.tile_pool(name="phpool", bufs=3) as pp,
            tc.tile_pool(name="psum", bufs=2, space="PSUM") as ps,
            tc.tile_pool(name="psacc", bufs=1, space="PSUM") as psacc,
        ):
            identb = cp.tile([128, 128], BF)
            make_identity(nc, identb[:])
            identf = cp.tile([128, 128], FP32)
            make_identity(nc, identf[:])
            ones64 = cp.tile([128, HID], BF)
            nc.vector.memset(ones64[:], 1.0)
            ones1 = cp.tile([128, 1], BF)
            nc.vector.memset(ones1[:], 1.0)
            w1t = cp.tile([F_IN, HID], FP32)
            nc.sync.dma_start(out=w1t[:], in_=w1p[:])
            b1r = cp.tile([128, HID], FP32)
            nc.sync.dma_start(out=b1r[:], in_=b1p[:].partition_broadcast(128))
            w2t = cp.tile([HID, N_CLASSES], FP32)
            nc.sync.dma_start(out=w2t[:], in_=w2p[:])
            w2b = cp.tile([HID, N_CLASSES], BF)
            nc.vector.tensor_copy(out=w2b[:], in_=w2t[:])
            b2t = cp.tile([N_CLASSES, 1], FP32)
            nc.sync.dma_start(out=b2t[:], in_=b2p[:])

            h = st.tile([128, NW * HID], FP32)
            x0s = st.tile([128, NW * HID], FP32)
            agg = st.tile([128, NW * HID], FP32)
            gst = st.tile([128, NW * HID], BF)
            dis = st.tile([128, NW], FP32)
            disA = st.tile([128, NW], FP32)

            # ---- h0 = relu(x @ W1 + b1) ----
            for wd in range(NW):
                xt = sp.tile([128, F_IN], FP32, tag="xt")
                nc.sync.dma_start(out=xt[:], in_=xsh[wd * 128 : (wd + 1) * 128, :])
                pxT = ps.tile([128, 128], FP32, space="PSUM", tag="p1")
                nc.tensor.transpose(out=pxT[:], in_=xt[:], identity=identf[:])
                xTs = sp.tile([128, 128], FP32, tag="xTs")
                nc.vector.tensor_copy(out=xTs[:], in_=pxT[:])
                ph0 = ps.tile([128, HID], FP32, space="PSUM", tag="p2")
                nc.tensor.matmul(out=ph0[:], lhsT=xTs[:], rhs=w1t[:], start=True, stop=True)
                tb = sp.tile([128, HID], FP32, tag="tb")
                nc.vector.tensor_add(out=tb[:], in0=ph0[:], in1=b1r[:])
                nc.scalar.activation(
                    out=h[:, wd * HID : (wd + 1) * HID],
                    in_=tb[:],
                    func=mybir.ActivationFunctionType.Relu,
                )
            nc.vector.tensor_scalar_mul(x0s[:], h[:], ALPHA)

            PB = 12  # phases per metadata block

            def agg_pass(use_gather):
                # token chunks for this pass; idx DMAs batched 8 chunks at a time
                chunk_tiles = {}
                if use_gather:
                    for cb in range(0, len(chunks), 8):
                        grp = chunks[cb : cb + 8]
                        gt0 = grp[0][0]
                        gtn = grp[-1][0] + grp[-1][1] - gt0
                        idxt = gp.tile([128, 8 * MAXCHUNK * 8], mybir.dt.int16, tag="idxt")
                        nc.scalar.dma_start(
                            out=idxt[:, : gtn * 8],
                            in_=idxp[:, gt0 * 8 : (gt0 + gtn) * 8],
                        )
                        for ci, (tile0, nt) in enumerate(grp):
                            rho = next(r for (r, ww, T_, t0) in phases if t0 <= tile0 < t0 + T_)
                            tok = gp.tile([128, MAXCHUNK, HID], BF, tag="tok")
                            nc.gpsimd.dma_gather(
                                out_ap=tok[:, :nt, :],
                                in_ap=table[rho * NRANGE : (rho + 1) * NRANGE, 0:HID],
                                idxs_ap=idxt[:, (tile0 - gt0) * 8 : (tile0 - gt0 + nt) * 8],
                                num_idxs=nt * 128,
                                num_idxs_reg=nt * 128,
                                elem_size=HID,
                                elem_step=128,
                                queue_num=(cb + ci) % 4,
                            )
                            for k in range(nt):
                                chunk_tiles[tile0 + k] = (tok, k)
                for pb in range(0, len(phases), PB):
                    blk = phases[pb : pb + PB]
                    nb = len(blk)
                    bt0 = blk[0][3]
                    btn = blk[-1][3] + blk[-1][2] - bt0
                    gwt = pp.tile([128, PB * 10 * SPAN], BF, tag="gwt")
                    nc.sync.dma_start(
                        out=gwt[:, : btn * SPAN],
                        in_=gwp[:, bt0 * SPAN : (bt0 + btn) * SPAN],
                    )
                    sat = pp.tile([128, PB, 128], BF, tag="sat")
                    nc.scalar.dma_start(
                        out=sat[:, :nb, :],
                        in_=sap[pb : pb + nb].rearrange("b p s -> p b s"),
                    )
                    sbt = pp.tile([32, PB, 128], BF, tag="sbt")
                    nc.sync.dma_start(
                        out=sbt[:, :nb, :],
                        in_=sbp[pb : pb + nb].rearrange("b p s -> p b s"),
                    )
                    for bi, (rho, ww, T_, tile0) in enumerate(blk):
                        nslot = T_ * SPAN
                        goff = (tile0 - bt0) * SPAN
                        p1 = ps.tile([HID, 10 * SPAN], FP32, space="PSUM", tag="p1")
                        for k in range(T_):
                            if use_gather:
                                tok, slot = chunk_tiles[tile0 + k]
                                lhs = tok[:, slot, :]
                            else:
                                lhs = ones64[:]
                            nc.tensor.matmul(
                                out=p1[:, 16 * k : 16 * (k + 1)],
                                lhsT=lhs,
                                rhs=gwt[:, goff + k * SPAN : goff + (k + 1) * SPAN],
                                start=True,
                                stop=True,
                            )
                        l1 = pp.tile([HID, 10 * SPAN], BF, tag="l1")
                        nc.vector.tensor_copy(out=l1[:, :nslot], in_=p1[:, :nslot])
                        n0 = min(nslot, 128)
                        t0p = ps.tile([128, HID], BF, space="PSUM", tag="t0p")
                        nc.tensor.transpose(out=t0p[:n0, :], in_=l1[:, :n0], identity=identb[:HID, :HID])
                        g0 = pp.tile([128, HID], BF, tag="g0")
                        nc.vector.tensor_copy(out=g0[:n0, :], in_=t0p[:n0, :])
                        p2 = ps.tile([128, HID], FP32, space="PSUM", tag="p2")
                        two = nslot > 128
                        nc.tensor.matmul(out=p2[:], lhsT=sat[:n0, bi, :], rhs=g0[:n0, :], start=True, stop=not two)
                        if two:
                            n1 = nslot - 128
                            t1p = ps.tile([32, HID], BF, space="PSUM", tag="t0p")
                            nc.tensor.transpose(out=t1p[:n1, :], in_=l1[:, 128:nslot], identity=identb[:HID, :HID])
                            g1 = pp.tile([32, HID], BF, tag="g1")
                            nc.vector.tensor_copy(out=g1[:n1, :], in_=t1p[:n1, :])
                            nc.tensor.matmul(out=p2[:], lhsT=sbt[:n1, bi, :], rhs=g1[:n1, :], start=False, stop=True)
                        dstv = agg[:, ww * HID : (ww + 1) * HID]
                        if rho == 0:
                            nc.vector.tensor_copy(out=dstv, in_=p2[:])
                        else:
                            nc.vector.tensor_add(out=dstv, in0=dstv, in1=p2[:])

            # ---- degree pass (table == ones) ----
            agg_pass(use_gather=False)
            degp = st.tile([128, NW], FP32)
            nc.vector.tensor_scalar_add(degp[:], agg[:].rearrange("p (w d) -> p w d", d=HID)[:, :, 0], 1.0)
            srt = st.tile([128, NW], FP32)
            nc.scalar.activation(out=srt[:], in_=degp[:], func=mybir.ActivationFunctionType.Sqrt)
            nc.vector.reciprocal(dis[:], srt[:])
            nc.vector.tensor_scalar_mul(disA[:], dis[:], 1.0 - ALPHA)

            disb = dis[:].unsqueeze(2).to_broadcast([128, NW, HID])
            disAb = disA[:].unsqueeze(2).to_broadcast([128, NW, HID])
            h3 = h[:].rearrange("p (w d) -> p w d", d=HID)
            gst3 = gst[:].rearrange("p (w d) -> p w d", d=HID)
            agg3 = agg[:].rearrange("p (w d) -> p w d", d=HID)

            # ---- K propagation steps ----
            for step in range(K):
                # gst = dis * h (bf16); write to table shard; AllGather
                nc.vector.tensor_tensor(out=gst3, in0=h3, in1=disb, op=mybir.AluOpType.mult)
                nc.sync.dma_start(
                    out=tin[:].rearrange("(w p) e -> p w e", p=128)[:, :, 0:HID],
                    in_=gst3,
                )
                nc.gpsimd.collective_compute(
                    "AllGather",
                    mybir.AluOpType.bypass,
                    replica_groups=RG,
                    ins=[tin[:]],
                    outs=[table[:]],
                )
                agg_pass(use_gather=True)
                # h = disA * (agg + gst) + x0s
                nc.vector.tensor_add(out=agg[:], in0=agg[:], in1=gst[:])
                nc.vector.tensor_tensor(out=agg3, in0=agg3, in1=disAb, op=mybir.AluOpType.mult)
                nc.vector.tensor_add(out=h[:], in0=agg[:], in1=x0s[:])

            # ---- pooling ----
            ppool = psacc.tile([HID + 1, N_GRAPHS], FP32, space="PSUM")
            for wd in range(NW):
                hc = sp.tile([128, HID], BF, tag="hc")
                nc.vector.tensor_copy(out=hc[:], in_=h[:, wd * HID : (wd + 1) * HID])
                poht = sp.tile([128, N_GRAPHS], BF, tag="poht")
                nc.sync.dma_start(out=poht[:], in_=pohp[:, wd * N_GRAPHS : (wd + 1) * N_GRAPHS])
                nc.tensor.matmul(
                    out=ppool[0:HID, :], lhsT=hc[:], rhs=poht[:],
                    start=(wd == 0), stop=(wd == NW - 1), skip_group_check=True,
                )
                nc.tensor.matmul(
                    out=ppool[HID : HID + 1, :], lhsT=ones1[:], rhs=poht[:],
                    start=(wd == 0), stop=(wd == NW - 1), skip_group_check=True,
                )
            psums = sp.tile([HID + 1, N_GRAPHS], FP32)
            nc.vector.tensor_copy(out=psums[:], in_=ppool[:])
            nc.sync.dma_start(out=arin[:], in_=psums[:])
            nc.gpsimd.collective_compute(
                "AllReduce", mybir.AluOpType.add, replica_groups=RG,
                ins=[arin[:]], outs=[arout[:]],
            )
            sums = sp.tile([HID, N_GRAPHS], FP32)
            nc.sync.dma_start(out=sums[:], in_=arout[0:HID, :])
            cnt = sp.tile([1, N_GRAPHS], FP32)
            nc.sync.dma_start(out=cnt[:], in_=arout[HID : HID + 1, :])
            nc.vector.tensor_scalar_max(cnt[:], cnt[:], 1.0)
            recip = sp.tile([1, N_GRAPHS], FP32)
            nc.vector.reciprocal(recip[:], cnt[:])
            nc.sync.dma_start(out=rtmp[:], in_=recip[:])
            recip10 = sp.tile([N_CLASSES, N_GRAPHS], FP32)
            nc.sync.dma_start(out=recip10[:], in_=rtmp[:].partition_broadcast(N_CLASSES))
            sumsb = sp.tile([HID, N_GRAPHS], BF)
            nc.vector.tensor_copy(out=sumsb[:], in_=sums[:])
            plg = ps.tile([N_CLASSES, N_GRAPHS], FP32, space="PSUM", tag="p1")
            nc.tensor.matmul(out=plg[:], lhsT=w2b[:], rhs=sumsb[:], start=True, stop=True)
            lgT = sp.tile([N_CLASSES, N_GRAPHS], FP32)
            nc.vector.tensor_tensor(out=lgT[:], in0=plg[:], in1=recip10[:], op=mybir.AluOpType.mult)
            nc.vector.tensor_scalar_add(lgT[:], lgT[:], b2t[:])
            # transpose to [512, 10] in 4 chunks of 128
            logit = sp.tile([128, 4, N_CLASSES], FP32)
            for k in range(4):
                ptr = ps.tile([128, N_CLASSES], FP32, space="PSUM", tag="p2")
                nc.tensor.transpose(
                    out=ptr[:], in_=lgT[:, 128 * k : 128 * (k + 1)],
                    identity=identf[:N_CLASSES, :N_CLASSES],
                )
                nc.vector.tensor_copy(out=logit[:, k, :], in_=ptr[:])
            # log_softmax along free axis (classes)
            m = sp.tile([128, 4], FP32)
            nc.vector.tensor_reduce(
                out=m[:], in_=logit[:], axis=mybir.AxisListType.X, op=mybir.AluOpType.max,
            )
            tshift = sp.tile([128, 4, N_CLASSES], FP32)
            nc.vector.tensor_tensor(
                out=tshift[:],
                in0=logit[:],
                in1=m[:].unsqueeze(2).to_broadcast([128, 4, N_CLASSES]),
                op=mybir.AluOpType.subtract,
            )
            ex = sp.tile([128, 4, N_CLASSES], FP32)
            nc.scalar.activation(out=ex[:], in_=tshift[:], func=mybir.ActivationFunctionType.Exp)
            s = sp.tile([128, 4], FP32)
            nc.vector.tensor_reduce(out=s[:], in_=ex[:], axis=mybir.AxisListType.X, op=mybir.AluOpType.add)
            ls = sp.tile([128, 4], FP32)
            nc.scalar.activation(out=ls[:], in_=s[:], func=mybir.ActivationFunctionType.Ln)
            outt = sp.tile([128, 4, N_CLASSES], FP32)
            nc.vector.tensor_tensor(
                out=outt[:],
                in0=tshift[:],
                in1=ls[:].unsqueeze(2).to_broadcast([128, 4, N_CLASSES]),
                op=mybir.AluOpType.subtract,
            )
            nc.sync.dma_start(
                out=outp[:].rearrange("(w p) c -> p w c", p=128),
                in_=outt[:],
            )

    nc.finalize()
    return nc


def _ensure_hooks():
    import antenv

    if "antenv.axon_hooks" in sys.modules:
        return
    m = types.ModuleType("antenv.axon_hooks")
    m._hook = None
    m.set_axon_ntff_profile_hook = lambda h: setattr(m, "_hook", h)
    m.get_axon_ntff_profile_hook = lambda: m._hook
    sys.modules["antenv.axon_hooks"] = m
    antenv.axon_hooks = m
    try:
        from trn_agent_boot.trn_boot import _ntff_profile_via_ctypes

        m._hook = _ntff_profile_via_ctypes("/opt/axon/libaxon_pjrt.so")
    except Exception:
        pass


def kernel(x, edge_index, edge_weight, batch, W1, b1, W2, b2, _trace=False):
    _ensure_hooks()
    from concourse.bass_utils import run_bass_kernel_spmd

    x = np.asarray(x, dtype=np.float32)
    W1 = np.asarray(W1, dtype=np.float32)
    b1 = np.asarray(b1, dtype=np.float32)
    W2 = np.asarray(W2, dtype=np.float32)
    b2 = np.asarray(b2, dtype=np.float32)

    key = "prog"
    if key not in _CACHE:
        meta, arrays = _build_structures(edge_index, edge_weight, batch)
        nc = _build_program(meta)
        _CACHE[key] = (meta, arrays, nc)
    meta, arrays, nc = _CACHE[key]

    in_maps = []
    for c in range(NC_):
        xs = np.zeros((NPCP, F_IN), np.float32)
        xs[:NPC] = x[c * NPC : (c + 1) * NPC]
        in_maps.append(
            dict(
                xsh=xs,
                idxp=arrays["idx"][c],
                gwp=arrays["gw"][c],
                sap=arrays["sa"][c],
                sbp=arrays["sb"][c],
                pohp=arrays["poh"][c],
                w1p=W1,
                b1p=b1.reshape(1, HID),
                w2p=W2,
                b2p=b2.reshape(N_CLASSES, 1),
            )
        )
    res = run_bass_kernel_spmd(nc, in_maps, list(range(NC_)), trace=_trace)
    out = res.results[0]["out"]
    if _trace:
        kernel.last_exec_ns = res.exec_time_ns
    return out


# revision 11
# speedup vs baseline: 1.0124x; 1.0124x over previous
"""APPNP GNN kernel for 8 TRN2 NeuronCores (Bass/Tile).

Strategy: node-sharded (12500 nodes/core), edges partitioned by dst core.
Per propagation step: AllGather of g = dis*h (bf16 table, 256B rows), then
dma_gather of per-edge source rows, weighted segment-sum via two matmul
levels (transposed mm1 routes tokens->16-slot windows in PSUM free axis;
PE transpose; mm2 routes slots->window rows with host-built selector
matrices), accumulated into SBUF agg. Degrees computed on device by the
same pipeline with an all-ones rhs. Epilogue pools by graph id via matmul
with a host-built one-hot, AllReduce, linear layer + log_softmax.
"""
import sys
import types

sys.path.insert(0, "/opt/trn_rl_repo")

import numpy as np

N = 100000
E = 3200000
F_IN = 128
HID = 64
N_CLASSES = 10
N_GRAPHS = 512
K = 5
ALPHA = 0.2
NC_ = 8
NPC = N // NC_          # 12500 nodes per core
NW = 98                 # windows of 128 rows
NPCP = NW * 128         # 12544 padded rows per core
NRANGE = 2 * NPCP       # 25088 rows per int16 index range
NROWS = NC_ * NPCP      # 100352 table rows
SPAN = 16               # max distinct nodes per 128-token tile
MAXCHUNK = 8            # tiles per gather call (<=1024 idxs)

_CACHE = {}


def _build_structures(edge_index, edge_weight, batch):
    import ml_dtypes

    BF16 = ml_dtypes.bfloat16
    src = np.asarray(edge_index[0], dtype=np.int64)
    dst = np.asarray(edge_index[1], dtype=np.int64)
    w = np.asarray(edge_weight, dtype=np.float32)
    batch = np.asarray(batch, dtype=np.int64)

    HALF = NPCP // 2
    _c = src // NPC
    _l = src % NPC
    prow = np.where(_l < HALF, _c * HALF + _l, NC_ * HALF + _c * HALF + (_l - HALF))
    rho_all = prow // NRANGE
    idx16_all = (prow - rho_all * NRANGE).astype(np.int16)
    core_all = dst // NPC
    ldst_all = (dst - core_all * NPC).astype(np.int64)

    # per (core, rho, win): tile lists with span<=SPAN rule
    per_core = []
    tiles_need = np.zeros((NC_, 4, NW), np.int64)
    for c in range(NC_):
        sel = np.nonzero(core_all == c)[0]
        ld = ldst_all[sel]
        rh = rho_all[sel]
        wi = ld // 128
        order = np.lexsort((ld, wi, rh))
        sel = sel[order]
        ld = ld[order]
        rh = rh[order]
        wi = wi[order]
        # segment boundaries for (rho, win)
        key = rh * NW + wi
        bounds = np.nonzero(np.diff(key))[0] + 1
        seg_starts = np.concatenate([[0], bounds])
        seg_ends = np.concatenate([bounds, [len(key)]])
        segs = {}
        for s, e in zip(seg_starts, seg_ends):
            r, ww = int(rh[s]), int(wi[s])
            # tile walk: each tile: up to 128 tokens, node span < SPAN
            tl = []
            p = s
            while p < e:
                base = int(ld[p])
                lim = p + np.searchsorted(ld[p:e], base + SPAN, side="left")
                q = min(p + 128, int(lim), e)
                tl.append((p, q, base))
                p = q
            segs[(r, ww)] = tl
            tiles_need[c, r, ww] = len(tl)
        per_core.append((sel, ld, segs))

    T = tiles_need.max(axis=0)  # [4, NW] uniform tiles per phase
    T = np.maximum(T, 1)
    # schedule: phases in (rho, win) order
    phases = []  # (rho, win, ntiles, tile0)
    t0 = 0
    for r in range(4):
        for ww in range(NW):
            phases.append((r, ww, int(T[r, ww]), t0))
            t0 += int(T[r, ww])
    ntiles = t0
    ntok = ntiles * 128
    # gather chunks per rho (cannot cross rho boundary)
    chunks = []  # (tile0, ntiles)
    for r in range(4):
        a = sum(int(T[rr, ww]) for rr in range(r) for ww in range(NW))
        b = a + sum(int(T[r, ww]) for ww in range(NW))
        p = a
        while p < b:
            nt = min(MAXCHUNK, b - p)
            chunks.append((p, nt))
            p += nt

    # per-core token arrays
    idx_rep_all, gw_all, sa_all, sb_all = [], [], [], []
    for c in range(NC_):
        sel, ld, segs = per_core[c]
        tok_idx = np.zeros(ntok, np.int16)
        tok_w = np.zeros(ntok, np.float32)
        tok_m = np.zeros(ntok, np.int64)
        tile_base = np.zeros(ntiles, np.int64)  # window-local base row of tile
        for r, ww, nt, tile0 in phases:
            tl = segs.get((r, ww), [])
            for k in range(nt):
                gt = tile0 + k
                if k < len(tl):
                    s, e, base = tl[k]
                    n = e - s
                    pos = gt * 128 + np.arange(n)
                    eidx = sel[s:e]
                    tok_idx[pos] = idx16_all[eidx]
                    tok_w[pos] = w[eidx]
                    tok_m[pos] = ld[s:e] - base
                    tile_base[gt] = base - ww * 128
                else:
                    tile_base[gt] = 0
        # wrapped idx layout [16, ntok/16] replicated x8
        iw = tok_idx.reshape(ntok // 16, 16).T
        idx_rep = np.broadcast_to(iw[None], (8, 16, ntok // 16)).reshape(128, ntok // 16)
        idx_rep_all.append(np.ascontiguousarray(idx_rep))
        # G_w [ntiles, 128, 16]
        gw = np.zeros((ntiles, 128, SPAN), np.float32)
        allpos = np.arange(ntok)
        gw[allpos // 128, allpos % 128, tok_m] = tok_w
        gw_all.append(np.ascontiguousarray(gw.transpose(1, 0, 2).reshape(128, ntiles * SPAN)).astype(BF16))
        # S matrices per phase: SA [nph, 128, 128], SB [nph, 32, 128]
        nph = len(phases)
        sa = np.zeros((nph, 128, 128), np.float32)
        sb = np.zeros((nph, 32, 128), np.float32)
        for pi, (r, ww, nt, tile0) in enumerate(phases):
            for k in range(nt):
                gt = tile0 + k
                slot0 = 16 * k
                base = tile_base[gt]
                for m in range(SPAN):
                    row = base + m
                    if row < 128:
                        if slot0 + m < 128:
                            sa[pi, slot0 + m, row] = 1.0
                        else:
                            sb[pi, slot0 + m - 128, row] = 1.0
        sa_all.append(sa.astype(BF16))
        sb_all.append(sb.astype(BF16))

    # pooling one-hot per core: [128, NW*512]
    poh_all = []
    cnt_mask_all = []
    for c in range(NC_):
        g_ids = batch[c * NPC : (c + 1) * NPC]
        poh = np.zeros((NPCP, N_GRAPHS), np.float32)
        poh[np.arange(NPC), g_ids] = 1.0
        poh = poh.reshape(NW, 128, N_GRAPHS).transpose(1, 0, 2).reshape(128, NW * N_GRAPHS)
        poh_all.append(poh.astype(BF16))

    meta = dict(phases=phases, chunks=chunks, ntiles=ntiles, ntok=ntok)
    arrays = dict(idx=idx_rep_all, gw=gw_all, sa=sa_all, sb=sb_all, poh=poh_all)
    return meta, arrays


def _patch_bass_elem128():
    import concourse.bass as bassmod

    if getattr(bassmod, "_elem128_patched", False):
        return
    src = open(bassmod.__file__).read().replace(
        "elem_size_bytes > 0 and elem_size_bytes % 256 == 0",
        "elem_size_bytes > 0 and elem_size_bytes % 128 == 0",
    )
    exec(compile(src, bassmod.__file__, "exec"), bassmod.__dict__)
    bassmod._elem128_patched = True


def _build_program(meta):
    import ml_dtypes

    _patch_bass_elem128()
    from concourse import bass, bacc, mybir
    from concourse.tile import TileContext
    from concourse.masks import make_identity

    phases = meta["phases"]
    chunks = meta["chunks"]
    ntiles = meta["ntiles"]
    ntok = meta["ntok"]
    nph = len(phases)
    FP32 = mybir.dt.float32
    BF = mybir.dt.bfloat16

    nc = bacc.Bacc("TRN2", num_swdge_queues=4)
    xsh = nc.declare_dram_parameter("xsh", [NPCP, F_IN], FP32, isOutput=False)
    idxp = nc.declare_dram_parameter("idxp", [128, ntok // 16], mybir.dt.int16, isOutput=False)
    gwp = nc.declare_dram_parameter("gwp", [128, ntiles * SPAN], BF, isOutput=False)
    sap = nc.declare_dram_parameter("sap", [nph, 128, 128], BF, isOutput=False)
    sbp = nc.declare_dram_parameter("sbp", [nph, 32, 128], BF, isOutput=False)
    pohp = nc.declare_dram_parameter("pohp", [128, NW * N_GRAPHS], BF, isOutput=False)
    w1p = nc.declare_dram_parameter("w1p", [F_IN, HID], FP32, isOutput=False)
    b1p = nc.declare_dram_parameter("b1p", [1, HID], FP32, isOutput=False)
    w2p = nc.declare_dram_parameter("w2p", [HID, N_CLASSES], FP32, isOutput=False)
    b2p = nc.declare_dram_parameter("b2p", [N_CLASSES, 1], FP32, isOutput=False)
    outp = nc.declare_dram_parameter("out", [N_GRAPHS, N_CLASSES], FP32, isOutput=True)

    tin = nc.dram_tensor("tin", [NPCP, 128], BF)
    table = nc.dram_tensor("table", [NROWS, 128], BF, addr_space="Shared")
    arin = nc.dram_tensor("arin", [HID + 1, N_GRAPHS], FP32)
    arout = nc.dram_tensor("arout", [HID + 1, N_GRAPHS], FP32, addr_space="Shared")
    rtmp = nc.dram_tensor("rtmp", [1, N_GRAPHS], FP32)

    RG = [list(range(NC_))]

    with TileContext(nc) as tc:
        with (
            tc.tile_pool(name="const", bufs=1) as cp,
            tc.tile_pool(name="state", bufs=1) as st,
            tc.tile_pool(name="stream", bufs=3) as sp,
            tc.tile_pool(name="gather", bufs=6) as gp,
            tc.tile_pool(name="phpool", bufs=3) as pp,
            tc.tile_pool(name="psum", bufs=2, space="PSUM") as ps,
            tc.tile_pool(name="psacc", bufs=1, space="PSUM") as psacc,
        ):
            identb = cp.tile([128, 128], BF)
            make_identity(nc, identb[:])
            identf = cp.tile([128, 128], FP32)
            make_identity(nc, identf[:])
            ones64 = cp.tile([128, HID], BF)
            nc.vector.memset(ones64[:], 1.0)
            ones1 = cp.tile([128, 1], BF)
            nc.vector.memset(ones1[:], 1.0)
            w1t = cp.tile([F_IN, HID], FP32)
            nc.sync.dma_start(out=w1t[:], in_=w1p[:])
            b1r = cp.tile([128, HID], FP32)
            nc.sync.dma_start(out=b1r[:], in_=b1p[:].partition_broadcast(128))
            w2t = cp.tile([HID, N_CLASSES], FP32)
            nc.sync.dma_start(out=w2t[:], in_=w2p[:])
            w2b = cp.tile([HID, N_CLASSES], BF)
            nc.vector.tensor_copy(out=w2b[:], in_=w2t[:])
            b2t = cp.tile([N_CLASSES, 1], FP32)
            nc.sync.dma_start(out=b2t[:], in_=b2p[:])

            h = st.tile([128, NW * HID], FP32)
            x0s = st.tile([128, NW * HID], FP32)
            agg = st.tile([128, NW * HID], FP32)
            gst = st.tile([128, NW * HID], BF)
            dis = st.tile([128, NW], FP32)
            disA = st.tile([128, NW], FP32)

            # ---- h0 = relu(x @ W1 + b1) ----
            for wd in range(NW):
                xt = sp.tile([128, F_IN], FP32, tag="xt")
                nc.sync.dma_start(out=xt[:], in_=xsh[wd * 128 : (wd + 1) * 128, :])
                pxT = ps.tile([128, 128], FP32, space="PSUM", tag="p1")
                nc.tensor.transpose(out=pxT[:], in_=xt[:], identity=identf[:])
                xTs = sp.tile([128, 128], FP32, tag="xTs")
                nc.vector.tensor_copy(out=xTs[:], in_=pxT[:])
                ph0 = ps.tile([128, HID], FP32, space="PSUM", tag="p2")
                nc.tensor.matmul(out=ph0[:], lhsT=xTs[:], rhs=w1t[:], start=True, stop=True)
                tb = sp.tile([128, HID], FP32, tag="tb")
                nc.vector.tensor_add(out=tb[:], in0=ph0[:], in1=b1r[:])
                nc.scalar.activation(
                    out=h[:, wd * HID : (wd + 1) * HID],
                    in_=tb[:],
                    func=mybir.ActivationFunctionType.Relu,
                )
            nc.vector.tensor_scalar_mul(x0s[:], h[:], ALPHA)

            PB = 8  # phases per metadata block

            def agg_pass(use_gather):
                # token chunks for this pass; idx DMAs batched 8 chunks at a time
                chunk_tiles = {}
                if use_gather:
                    for cb in range(0, len(chunks), 8):
                        grp = chunks[cb : cb + 8]
                        gt0 = grp[0][0]
                        gtn = grp[-1][0] + grp[-1][1] - gt0
                        idxt = gp.tile([128, 8 * MAXCHUNK * 8], mybir.dt.int16, tag="idxt")
                        nc.scalar.dma_start(
                            out=idxt[:, : gtn * 8],
                            in_=idxp[:, gt0 * 8 : (gt0 + gtn) * 8],
                        )
                        for ci, (tile0, nt) in enumerate(grp):
                            rho = next(r for (r, ww, T_, t0) in phases if t0 <= tile0 < t0 + T_)
                            tok = gp.tile([128, MAXCHUNK, HID], BF, tag="tok")
                            nc.gpsimd.dma_gather(
                                out_ap=tok[:, :nt, :],
                                in_ap=table[rho * NRANGE : (rho + 1) * NRANGE, 0:HID],
                                idxs_ap=idxt[:, (tile0 - gt0) * 8 : (tile0 - gt0 + nt) * 8],
                                num_idxs=nt * 128,
                                num_idxs_reg=nt * 128,
                                elem_size=HID,
                                elem_step=128,
                                queue_num=(cb + ci) % 4,
                            )
                            for k in range(nt):
                                chunk_tiles[tile0 + k] = (tok, k)
                for pb in range(0, len(phases), PB):
                    blk = phases[pb : pb + PB]
                    nb = len(blk)
                    bt0 = blk[0][3]
                    btn = blk[-1][3] + blk[-1][2] - bt0
                    gwt = pp.tile([128, PB * 10 * SPAN], BF, tag="gwt")
                    nc.sync.dma_start(
                        out=gwt[:, : btn * SPAN],
                        in_=gwp[:, bt0 * SPAN : (bt0 + btn) * SPAN],
                    )
                    sat = pp.tile([128, PB, 128], BF, tag="sat")
                    nc.scalar.dma_start(
                        out=sat[:, :nb, :],
                        in_=sap[pb : pb + nb].rearrange("b p s -> p b s"),
                    )
                    sbt = pp.tile([32, PB, 128], BF, tag="sbt")
                    nc.sync.dma_start(
                        out=sbt[:, :nb, :],
                        in_=sbp[pb : pb + nb].rearrange("b p s -> p b s"),
                    )
                    for bi, (rho, ww, T_, tile0) in enumerate(blk):
                        nslot = T_ * SPAN
                        goff = (tile0 - bt0) * SPAN
                        p1 = ps.tile([HID, 10 * SPAN], FP32, space="PSUM", tag="p1")
                        for k in range(T_):
                            if use_gather:
                                tok, slot = chunk_tiles[tile0 + k]
                                lhs = tok[:, slot, :]
                            else:
                                lhs = ones64[:]
                            nc.tensor.matmul(
                                out=p1[:, 16 * k : 16 * (k + 1)],
                                lhsT=lhs,
                                rhs=gwt[:, goff + k * SPAN : goff + (k + 1) * SPAN],
                                start=True,
                                stop=True,
                            )
                        l1 = pp.tile([HID, 10 * SPAN], BF, tag="l1")
                        nc.vector.tensor_copy(out=l1[:, :nslot], in_=p1[:, :nslot])
                        n0 = min(nslot, 128)
                        t0p = ps.tile([128, HID], BF, space="PSUM", tag="t0p")
                        nc.tensor.transpose(out=t0p[:n0, :], in_=l1[:, :n0], identity=identb[:HID, :HID])
                        g0 = pp.tile([128, HID], BF, tag="g0")
                        nc.vector.tensor_copy(out=g0[:n0, :], in_=t0p[:n0, :])
                        p2 = ps.tile([128, HID], FP32, space="PSUM", tag="p2")
                        two = nslot > 128
                        nc.tensor.matmul(out=p2[:], lhsT=sat[:n0, bi, :], rhs=g0[:n0, :], start=True, stop=not two)
                        if two:
                            n1 = nslot - 128
                            t1p = ps.tile([32, HID], BF, space="PSUM", tag="t0p")
                            nc.tensor.transpose(out=t1p[:n1, :], in_=l1[:, 128:nslot], identity=identb[:HID, :HID])
                            g1 = pp.tile([32, HID], BF, tag="g1")
                            nc.vector.tensor_copy(out=g1[:n1, :], in_=t1p[:n1, :])
                            nc.tensor.matmul(out=p2[:], lhsT=sbt[:n1, bi, :], rhs=g1[:n1, :], start=False, stop=True)
                        dstv = agg[:, ww * HID : (ww + 1) * HID]
                        if rho == 0:
                            nc.vector.tensor_copy(out=dstv, in_=p2[:])
                        else:
                            nc.vector.tensor_add(out=dstv, in0=dstv, in1=p2[:])

            # ---- degree pass (table == ones) ----
            agg_pass(use_gather=False)
            degp = st.tile([128, NW], FP32)
            nc.vector.tensor_scalar_add(degp[:], agg[:].rearrange("p (w d) -> p w d", d=HID)[:, :, 0], 1.0)
            srt = st.tile([128, NW], FP32)
            nc.scalar.activation(out=srt[:], in_=degp[:], func=mybir.ActivationFunctionType.Sqrt)
            nc.vector.reciprocal(dis[:], srt[:])
            nc.vector.tensor_scalar_mul(disA[:], dis[:], 1.0 - ALPHA)

            disb = dis[:].unsqueeze(2).to_broadcast([128, NW, HID])
            disAb = disA[:].unsqueeze(2).to_broadcast([128, NW, HID])
            h3 = h[:].rearrange("p (w d) -> p w d", d=HID)
            gst3 = gst[:].rearrange("p (w d) -> p w d", d=HID)
            agg3 = agg[:].rearrange("p (w d) -> p w d", d=HID)

            # ---- K propagation steps ----
            for step in range(K):
                # gst = dis * h (bf16); write to table shard; AllGather
                nc.vector.tensor_tensor(out=gst3, in0=h3, in1=disb, op=mybir.AluOpType.mult)
                nc.sync.dma_start(
                    out=tin[:].rearrange("(w p) e -> p w e", p=128)[:, :, 0:HID],
                    in_=gst3,
                )
                HALF = NPCP // 2
                nc.gpsimd.collective_compute(
                    "AllGather",
                    mybir.AluOpType.bypass,
                    replica_groups=RG,
                    ins=[tin[0:HALF, :]],
                    outs=[table[0 : NC_ * HALF, :]],
                )
                nc.gpsimd.collective_compute(
                    "AllGather",
                    mybir.AluOpType.bypass,
                    replica_groups=RG,
                    ins=[tin[HALF:NPCP, :]],
                    outs=[table[NC_ * HALF : NROWS, :]],
                )
                agg_pass(use_gather=True)
                # h = disA * (agg + gst) + x0s
                nc.vector.tensor_add(out=agg[:], in0=agg[:], in1=gst[:])
                nc.vector.tensor_tensor(out=agg3, in0=agg3, in1=disAb, op=mybir.AluOpType.mult)
                nc.vector.tensor_add(out=h[:], in0=agg[:], in1=x0s[:])

            # ---- pooling ----
            ppool = psacc.tile([HID + 1, N_GRAPHS], FP32, space="PSUM")
            for wd in range(NW):
                hc = sp.tile([128, HID], BF, tag="hc")
                nc.vector.tensor_copy(out=hc[:], in_=h[:, wd * HID : (wd + 1) * HID])
                poht = sp.tile([128, N_GRAPHS], BF, tag="poht")
                nc.sync.dma_start(out=poht[:], in_=pohp[:, wd * N_GRAPHS : (wd + 1) * N_GRAPHS])
                nc.tensor.matmul(
                    out=ppool[0:HID, :], lhsT=hc[:], rhs=poht[:],
                    start=(wd == 0), stop=(wd == NW - 1), skip_group_check=True,
                )
                nc.tensor.matmul(
                    out=ppool[HID : HID + 1, :], lhsT=ones1[:], rhs=poht[:],
                    start=(wd == 0), stop=(wd == NW - 1), skip_group_check=True,
                )
            psums = sp.tile([HID + 1, N_GRAPHS], FP32)
            nc.vector.tensor_copy(out=psums[:], in_=ppool[:])
            nc.sync.dma_start(out=arin[:], in_=psums[:])
            nc.gpsimd.collective_compute(
                "AllReduce", mybir.AluOpType.add, replica_groups=RG,
                ins=[arin[:]], outs=[arout[:]],
            )
            sums = sp.tile([HID, N_GRAPHS], FP32)
            nc.sync.dma_start(out=sums[:], in_=arout[0:HID, :])
            cnt = sp.tile([1, N_GRAPHS], FP32)
            nc.sync.dma_start(out=cnt[:], in_=arout[HID : HID + 1, :])
            nc.vector.tensor_scalar_max(cnt[:], cnt[:], 1.0)
            recip = sp.tile([1, N_GRAPHS], FP32)
            nc.vector.reciprocal(recip[:], cnt[:])
            nc.sync.dma_start(out=rtmp[:], in_=recip[:])
            recip10 = sp.tile([N_CLASSES, N_GRAPHS], FP32)
            nc.sync.dma_start(out=recip10[:], in_=rtmp[:].partition_broadcast(N_CLASSES))
            sumsb = sp.tile([HID, N_GRAPHS], BF)
            nc.vector.tensor_copy(out=sumsb[:], in_=sums[:])
            plg = ps.tile([N_CLASSES, N_GRAPHS], FP32, space="PSUM", tag="p1")
            nc.tensor.matmul(out=plg[:], lhsT=w2b[:], rhs=sumsb[:], start=True, stop=True)
            lgT = sp.tile([N_CLASSES, N_GRAPHS], FP32)
            nc.vector.tensor_tensor(out=lgT[:], in0=plg[:], in1=recip10[:], op=mybir.AluOpType.mult)
            nc.vector.tensor_scalar_add(lgT[:], lgT[:], b2t[:])
            # transpose to [512, 10] in 4 chunks of 128
            logit = sp.tile([128, 4, N_CLASSES], FP32)
            for k in range(4):
                ptr = ps.tile([128, N_CLASSES], FP32, space="PSUM", tag="p2")
                nc.tensor.transpose(
                    out=ptr[:], in_=lgT[:, 128 * k : 128 * (k + 1)],
                    identity=identf[:N_CLASSES, :N_CLASSES],
                )
                nc.vector.tensor_copy(out=logit[:, k, :], in_=ptr[:])
            # log_softmax along free axis (classes)
            m = sp.tile([128, 4], FP32)
            nc.vector.tensor_reduce(
                out=m[:], in_=logit[:], axis=mybir.AxisListType.X, op=mybir.AluOpType.max,
            )
            tshift = sp.tile([128, 4, N_CLASSES], FP32)
            nc.vector.tensor_tensor(
                out=tshift[:],
                in0=logit[:],
                in1=m[:].unsqueeze(2).to_broadcast([128, 4, N_CLASSES]),
                op=mybir.AluOpType.subtract,
            )
            ex = sp.tile([128, 4, N_CLASSES], FP32)
            nc.scalar.activation(out=ex[:], in_=tshift[:], func=mybir.ActivationFunctionType.Exp)
            s = sp.tile([128, 4], FP32)
            nc.vector.tensor_reduce(out=s[:], in_=ex[:], axis=mybir.AxisListType.X, op=mybir.AluOpType.add)
            ls = sp.tile([128, 4], FP32)
            nc.scalar.activation(out=ls[:], in_=s[:], func=mybir.ActivationFunctionType.Ln)
            outt = sp.tile([128, 4, N_CLASSES], FP32)
            nc.vector.tensor_tensor(
                out=outt[:],
                in0=tshift[:],
                in1=ls[:].unsqueeze(2).to_broadcast([128, 4, N_CLASSES]),
                op=mybir.AluOpType.subtract,
            )
            nc.sync.dma_start(
                out=outp[:].rearrange("(w p) c -> p w c", p=128),
                in_=outt[:],
            )

    nc.finalize()
    return nc


def _ensure_hooks():
    import antenv

    if "antenv.axon_hooks" in sys.modules:
        return
    m = types.ModuleType("antenv.axon_hooks")
    m._hook = None
    m.set_axon_ntff_profile_hook = lambda h: setattr(m, "_hook", h)
    m.get_axon_ntff_profile_hook = lambda: m._hook
    sys.modules["antenv.axon_hooks"] = m
    antenv.axon_hooks = m
    try:
        from trn_agent_boot.trn_boot import _ntff_profile_via_ctypes

        m._hook = _ntff_profile_via_ctypes("/opt/axon/libaxon_pjrt.so")
    except Exception:
        pass


def kernel(x, edge_index, edge_weight, batch, W1, b1, W2, b2, _trace=False):
    _ensure_hooks()
    from concourse.bass_utils import run_bass_kernel_spmd

    x = np.asarray(x, dtype=np.float32)
    W1 = np.asarray(W1, dtype=np.float32)
    b1 = np.asarray(b1, dtype=np.float32)
    W2 = np.asarray(W2, dtype=np.float32)
    b2 = np.asarray(b2, dtype=np.float32)

    key = "prog"
    if key not in _CACHE:
        meta, arrays = _build_structures(edge_index, edge_weight, batch)
        nc = _build_program(meta)
        _CACHE[key] = (meta, arrays, nc)
    meta, arrays, nc = _CACHE[key]

    in_maps = []
    for c in range(NC_):
        xs = np.zeros((NPCP, F_IN), np.float32)
        xs[:NPC] = x[c * NPC : (c + 1) * NPC]
        in_maps.append(
            dict(
                xsh=xs,
                idxp=arrays["idx"][c],
                gwp=arrays["gw"][c],
                sap=arrays["sa"][c],
                sbp=arrays["sb"][c],
                pohp=arrays["poh"][c],
                w1p=W1,
                b1p=b1.reshape(1, HID),
                w2p=W2,
                b2p=b2.reshape(N_CLASSES, 1),
            )
        )
    res = run_bass_kernel_spmd(nc, in_maps, list(range(NC_)), trace=_trace)
    out = res.results[0]["out"]
    if _trace:
        kernel.last_exec_ns = res.exec_time_ns
    return out
